# revision 1
# baseline (speedup 1.0000x reference)
"""BuildingGCN Trainium2 kernel: 3-layer GCN + global mean pool + MLP head,
distributed over 8 NeuronCores.

Strategy per core (node shard = 12500 nodes, edges sharded by dst owner):
  y = dinv*(h@W) table in DRAM (AllGathered across cores each layer);
  per 128-dst-node window: dma_gather y[src] for dst-sorted edges (4 int16
  chunk tables), segment-sum via band-matrix matmuls on TensorE (PSUM
  accumulation), post-ops h = relu(dinv*S + invdeg*xw + b) on DVE.
  Band matrices generated on-device: is_equal(seg, iota); pad slots seg=-1.
  Pool: band matmul -> AllGather -> host-known-offset adds; MLP replicated.
"""
import os
import sys
import types
from dataclasses import dataclass, field

import numpy as np

import concourse.bass as bass
import concourse.tile as tile
from concourse import bacc, mybir
from concourse._compat import cdiv
from concourse.bass_utils import run_bass_kernel_spmd

P = 128
F32 = mybir.dt.float32
I16 = mybir.dt.int16


@dataclass
class Cfg:
    n_nodes: int = 100000
    n_cores: int = 8
    n_graphs: int = 256
    chunk: int = 25000          # gather table chunk rows (int16-safe)
    wpg: int = 2                # windows per gather group
    gslots: int = 64            # graph slots per core
    in_ch: int = 8
    c1: int = 64
    c2: int = 128
    c3: int = 64
    ch1: int = 32               # MLP hidden

    @property
    def ns(self):
        return self.n_nodes // self.n_cores

    @property
    def nw(self):
        return cdiv(self.ns, P)

    @property
    def n_chunks(self):
        return cdiv(self.n_nodes, self.chunk)

    @property
    def ng(self):
        return cdiv(self.nw, self.wpg)

    @property
    def w_last(self):
        return self.ns - (self.nw - 1) * P


@dataclass
class Plan:
    cfg: Cfg
    nbwk: np.ndarray          # [NW, n_chunks] blocks per (window, chunk), cross-core max
    g0: list = field(default_factory=list)     # per-core first graph id
    gw: list = field(default_factory=list)     # per-core graph width (<= gslots)
    # derived
    nb_g: list = field(default_factory=list)       # blocks per group
    kw_off: list = field(default_factory=list)     # [g][k] block offset of call (g,k) in group tile
    call_ni: list = field(default_factory=list)    # [g][k] num idxs (0 => skip)
    icol0: list = field(default_factory=list)      # [g][k] idx_all col offset
    b0_g: list = field(default_factory=list)       # seg_all block offset of group
    blk_rows: list = field(default_factory=list)   # [g][w_local] -> list of block rows j in group

    def finalize(self):
        cfg = self.cfg
        icol = 0
        b0 = 0
        for g in range(cfg.ng):
            ws = list(range(g * cfg.wpg, min((g + 1) * cfg.wpg, cfg.nw)))
            offs = []
            nis = []
            cols = []
            off = 0
            for k in range(cfg.n_chunks):
                offs.append(off)
                nb = int(sum(self.nbwk[w, k] for w in ws))
                nis.append(nb * P)
                cols.append(icol)
                icol += nb * P // 16
                off += nb
            self.kw_off.append(offs)
            self.call_ni.append(nis)
            self.icol0.append(cols)
            self.nb_g.append(off)
            self.b0_g.append(b0)
            b0 += off
            rows = []
            for wl, w in enumerate(ws):
                r = []
                for k in range(cfg.n_chunks):
                    base = offs[k] + int(sum(self.nbwk[w2, k] for w2 in ws[:wl]))
                    r.extend(range(base, base + int(self.nbwk[w, k])))
                rows.append(r)
            self.blk_rows.append(rows)
        self.idx_cols = icol
        self.nb_total = b0


def preprocess(cfg: Cfg, edge_index, batch):
    """Host-side graph partitioning. Returns (plan, shared dict, per-core dicts)."""
    src = np.asarray(edge_index[0], dtype=np.int64)
    dst = np.asarray(edge_index[1], dtype=np.int64)
    batch = np.asarray(batch, dtype=np.int64)
    N, NC, NS, NW = cfg.n_nodes, cfg.n_cores, cfg.ns, cfg.nw

    deg = (np.bincount(dst, minlength=N) + 1.0).astype(np.float32)
    dinv = (1.0 / np.sqrt(deg)).astype(np.float32)
    invdeg = (1.0 / deg).astype(np.float32)
    cnt = np.bincount(batch, minlength=cfg.n_graphs).astype(np.float32)
    invcnt = (1.0 / np.maximum(cnt, 1.0)).astype(np.float32)

    per_core_edges = []
    counts = np.zeros((NC, NW, cfg.n_chunks), dtype=np.int64)
    for c in range(NC):
        m = (dst >= c * NS) & (dst < (c + 1) * NS)
        s = src[m]
        d = dst[m] - c * NS
        w = d // P
        seg = (d % P).astype(np.float32)
        k = s // cfg.chunk
        sl = (s % cfg.chunk).astype(np.int16)
        key = k * NW + w
        order = np.argsort(key, kind="stable")
        s, seg, w, k, sl, key = s[order], seg[order], w[order], k[order], sl[order], key[order]
        cnts = np.bincount(key, minlength=NW * cfg.n_chunks)
        counts[c] = cnts.reshape(cfg.n_chunks, NW).T
        starts = np.zeros(NW * cfg.n_chunks + 1, dtype=np.int64)
        np.cumsum(cnts, out=starts[1:])
        per_core_edges.append((sl, seg, starts))

    nbwk = np.ceil(counts.max(axis=0) / P).astype(np.int64)  # [NW, n_chunks]
    plan = Plan(cfg, nbwk)
    plan.g0 = [int(batch[c * NS]) for c in range(NC)]
    plan.gw = [int(batch[(c + 1) * NS - 1]) - plan.g0[c] + 1 for c in range(NC)]
    assert max(plan.gw) <= cfg.gslots, f"graph width {max(plan.gw)} > {cfg.gslots}"
    plan.finalize()

    shared = {
        "iota_t": np.tile(np.arange(P, dtype=np.float32), (P, 1)),
        "invcnt_r": np.tile(invcnt, (cfg.c3, 1)).astype(np.float32),
    }

    cores = []
    for c in range(NC):
        sl, seg, starts = per_core_edges[c]
        idx_stream = np.zeros(plan.idx_cols * 16, dtype=np.int16)
        seg_stream = np.full(plan.nb_total * P, -1.0, dtype=np.float32)
        for g in range(cfg.ng):
            ws = list(range(g * cfg.wpg, min((g + 1) * cfg.wpg, cfg.nw)))
            for k in range(cfg.n_chunks):
                ipos = plan.icol0[g][k] * 16
                bpos = (plan.b0_g[g] + plan.kw_off[g][k]) * P
                for w in ws:
                    e0, e1 = starts[k * NW + w], starts[k * NW + w + 1]
                    n = e1 - e0
                    nslot = int(nbwk[w, k]) * P
                    idx_stream[ipos:ipos + n] = sl[e0:e1]
                    seg_stream[bpos:bpos + n] = seg[e0:e1]
                    ipos += nslot
                    bpos += nslot
        idx_all = np.tile(idx_stream.reshape(-1, 16).T, (8, 1))  # [128, idx_cols]
        seg_all = seg_stream.reshape(-1, P).T.copy()             # [128, nb_total]

        nodes = np.arange(c * NS, (c + 1) * NS)
        dinv_t = np.ones((P, NW), dtype=np.float32)
        invdeg_t = np.ones((P, NW), dtype=np.float32)
        gslot_t = np.full((P, NW), -1.0, dtype=np.float32)
        dv = dinv[nodes].reshape(-1)
        iv = invdeg[nodes].reshape(-1)
        gs = (batch[nodes] - plan.g0[c]).astype(np.float32)
        fl_d = dinv_t.T.reshape(-1)
        fl_d[:NS] = dv
        fl_i = invdeg_t.T.reshape(-1)
        fl_i[:NS] = iv
        fl_g = gslot_t.T.reshape(-1)
        fl_g[:NS] = gs
        dinv_t = fl_d.reshape(NW, P).T.copy()
        invdeg_t = fl_i.reshape(NW, P).T.copy()
        gslot_t = fl_g.reshape(NW, P).T.copy()
        cores.append({
            "idx_all": idx_all, "seg_all": seg_all,
            "dinv_t": dinv_t, "invdeg_t": invdeg_t, "gslot_t": gslot_t,
        })
    return plan, shared, cores


def build_program(plan: Plan, n_cores: int):
    cfg = plan.cfg
    NW, NS, NG = cfg.nw, cfg.ns, cfg.ng
    nc = bacc.Bacc("TRN2", target_bir_lowering=False, debug=False,
                   num_devices=n_cores, num_swdge_queues=4)

    def din(name, shape, dt=F32):
        return nc.dram_tensor(name, shape, dt, kind="ExternalInput").ap()

    xT = din("xT", [cfg.in_ch, NW * P])
    w1 = din("w1", [cfg.in_ch, cfg.c1])
    w2 = din("w2", [cfg.c1, cfg.c2])
    w3 = din("w3", [cfg.c2, cfg.c3])
    wl1 = din("wl1", [cfg.c3, cfg.ch1])
    wl2 = din("wl2", [cfg.ch1, 1])
    b1r = din("b1r", [P, cfg.c1])
    b2r = din("b2r", [P, cfg.c2])
    b3r = din("b3r", [P, cfg.c3])
    bl1c = din("bl1c", [cfg.ch1, 1])
    bl2c = din("bl2c", [1, 1])
    dinv_d = din("dinv_t", [P, NW])
    invdeg_d = din("invdeg_t", [P, NW])
    gslot_d = din("gslot_t", [P, NW])
    invcnt_d = din("invcnt_r", [cfg.c3, cfg.n_graphs])
    iota_d = din("iota_t", [P, P])
    ident_d = din("ident_t", [P, P])
    idx_d = din("idx_all", [P, plan.idx_cols], I16)
    seg_d = din("seg_all", [P, plan.nb_total])
    out_d = nc.dram_tensor("out", [1, cfg.n_graphs], F32, kind="ExternalOutput").ap()

    with tile.TileContext(nc) as tc:
        with tc.tile_pool(name="const", bufs=1) as cp, \
             tc.tile_pool(name="gath", bufs=2) as gp, \
             tc.tile_pool(name="band", bufs=2) as bp, \
             tc.tile_pool(name="win", bufs=3) as wp, \
             tc.tile_pool(name="psum", bufs=2, space="PSUM") as pp, \
             tc.tile_pool(name="psagg", bufs=4, space="PSUM") as ppa, \
             tc.tile_pool(name="psum1", bufs=1, space="PSUM") as pp1, \
             tc.tile_pool(name="dram", bufs=1, space="DRAM") as dp:

            def load_const(name, ap, shape, dt=F32):
                t = cp.tile(shape, dt, tag=name)
                nc.sync.dma_start(t[:], ap[:])
                return t

            w1_s = load_const("w1", w1, [cfg.in_ch, cfg.c1])
            w2_s = load_const("w2", w2, [cfg.c1, cfg.c2])
            w3_s = load_const("w3", w3, [cfg.c2, cfg.c3])
            wl1_s = load_const("wl1", wl1, [cfg.c3, cfg.ch1])
            wl2_s = load_const("wl2", wl2, [cfg.ch1, 1])
            b1_s = load_const("b1", b1r, [P, cfg.c1])
            b2_s = load_const("b2", b2r, [P, cfg.c2])
            b3_s = load_const("b3", b3r, [P, cfg.c3])
            bl1_s = load_const("bl1", bl1c, [cfg.ch1, 1])
            bl2_s = load_const("bl2", bl2c, [1, 1])
            dinv_s = load_const("dinv", dinv_d, [P, NW])
            invdeg_s = load_const("invdeg", invdeg_d, [P, NW])
            gslot_s = load_const("gslot", gslot_d, [P, NW])
            invcnt_s = load_const("invcnt", invcnt_d, [cfg.c3, cfg.n_graphs])
            iota_s = load_const("iota", iota_d, [P, P])
            ident_s = load_const("ident", ident_d, [P, P])

            pool_acc = cp.tile([cfg.c3, cfg.gslots], F32)
            nc.vector.memset(pool_acc[:], 0.0)

            ys64 = dp.tile([NS, cfg.c1], F32)
            ys128 = dp.tile([NS, cfg.c2], F32)
            yfull64 = dp.tile([cfg.n_nodes, cfg.c1], F32)
            yfull128 = dp.tile([cfg.n_nodes, cfg.c2], F32)
            xw_a = dp.tile([NW * P, cfg.c1], F32)
            xw_b = dp.tile([NW * P, cfg.c2], F32)
            pool_sh = dp.tile([cfg.c3, cfg.gslots], F32)
            pool_ag = dp.tile([cfg.c3 * n_cores, cfg.gslots], F32)

            rg = [list(range(n_cores))]

            def ts(out, in0, s1, op0, s2=None, op1=mybir.AluOpType.bypass):
                nc.vector.tensor_scalar(out, in0, s1, s2, op0, op1)

            def y_and_xw(ps_xw, w, cin_next, dinv_col, ys_dst, xw_dst, rows):
                """From xw psum: save xw to DRAM, y=dinv*xw to DRAM shard."""
                xw_t = wp.tile([P, cin_next], F32, tag="xwt")
                nc.vector.tensor_copy(xw_t[:], ps_xw[:])
                nc.sync.dma_start(xw_dst[w * P:(w + 1) * P, :], xw_t[:])
                y_t = wp.tile([P, cin_next], F32, tag="yt")
                ts(y_t[:], ps_xw[:], dinv_col, mybir.AluOpType.mult)
                nc.sync.dma_start(ys_dst[w * P:w * P + rows, :], y_t[:rows, :])

            # ---- loop 1: xw1 = x @ W1 ; y1 = dinv * xw1 ----
            for w in range(NW):
                rows = cfg.w_last if w == NW - 1 else P
                xt_w = wp.tile([cfg.in_ch, P], F32, tag="xtw")
                nc.sync.dma_start(xt_w[:], xT[:, w * P:(w + 1) * P])
                ps = pp.tile([P, cfg.c1], F32, tag="psxw")
                nc.tensor.matmul(ps[:], lhsT=xt_w[:],
                                 rhs=w1_s[:], start=True, stop=True)
                y_and_xw(ps, w, cfg.c1, dinv_s[:, w:w + 1], ys64, xw_a, rows)

            nc.gpsimd.collective_compute(
                "AllGather", mybir.AluOpType.bypass, replica_groups=rg,
                ins=[ys64.opt()], outs=[yfull64.opt()])

            # ---- aggregation layer ----
            def agg_layer(lnum, cin, cout, ytab, xw_cur, b_s, wnext_s,
                          ys_next, xw_next):
                for g in range(NG):
                    ws = list(range(g * cfg.wpg, min((g + 1) * cfg.wpg, NW)))
                    nbg = plan.nb_g[g]
                    if nbg == 0:
                        continue
                    gt = gp.tile([P, nbg, cin], F32, tag="gt")
                    for k in range(cfg.n_chunks):
                        ni = plan.call_ni[g][k]
                        if ni == 0:
                            continue
                        cols = ni // 16
                        it = wp.tile([P, cols], I16, tag="idxt")
                        c0 = plan.icol0[g][k]
                        nc.sync.dma_start(it[:], idx_d[:, c0:c0 + cols])
                        o = plan.kw_off[g][k]
                        nb = ni // P
                        rlo = k * cfg.chunk
                        rhi = min(rlo + cfg.chunk, cfg.n_nodes)
                        nc.gpsimd.dma_gather(
                            gt[:, o:o + nb, :], ytab[rlo:rhi, :], it[:],
                            ni, ni, cin, single_packet=False, queue_num=k)
                    band = bp.tile([P, nbg, P], F32, tag="band")
                    sg0 = plan.b0_g[g]
                    seg_t = wp.tile([P, nbg], F32, tag="segt")
                    nc.sync.dma_start(seg_t[:], seg_d[:, sg0:sg0 + nbg])
                    nc.vector.tensor_tensor(
                        out=band[:],
                        in0=seg_t[:].unsqueeze(2).broadcast_to([P, nbg, P]),
                        in1=iota_s[:].unsqueeze(1).broadcast_to([P, nbg, P]),
                        op=mybir.AluOpType.is_equal)
                    for wl, w in enumerate(ws):
                        rows = cfg.w_last if w == NW - 1 else P
                        brows = plan.blk_rows[g][wl]
                        ps = ppa.tile([P, cin], F32, tag="psagg")
                        if not brows:
                            nc.vector.memset(ps[:], 0.0)
                        for j, b in enumerate(brows):
                            nc.tensor.matmul(
                                ps[:], lhsT=band[:, b, :], rhs=gt[:, b, :],
                                start=(j == 0), stop=(j == len(brows) - 1),
                                skip_group_check=True)
                        # h = relu(dinv*S + invdeg*xw + bias)
                        xw_t = wp.tile([P, cin], F32, tag="xwin")
                        nc.sync.dma_start(xw_t[:], xw_cur[w * P:(w + 1) * P, :])
                        t1 = wp.tile([P, cin], F32, tag="t1")
                        ts(t1[:], ps[:], dinv_s[:, w:w + 1], mybir.AluOpType.mult)
                        t2 = wp.tile([P, cin], F32, tag="t2")
                        ts(t2[:], xw_t[:], invdeg_s[:, w:w + 1], mybir.AluOpType.mult)
                        t3 = wp.tile([P, cin], F32, tag="t3")
                        nc.vector.tensor_tensor(t3[:], t1[:], t2[:], mybir.AluOpType.add)
                        t4 = wp.tile([P, cin], F32, tag="t4")
                        nc.vector.tensor_tensor(t4[:], t3[:], b_s[:], mybir.AluOpType.add)
                        h = wp.tile([P, cin], F32, tag="h")
                        ts(h[:], t4[:], 0.0, mybir.AluOpType.max)
                        if lnum < 3:
                            ps_t = pp1.tile([cin, P], F32, tag="pst")
                            nc.tensor.transpose(ps_t[:], h[:], ident_s[:])
                            hT = wp.tile([cin, P], F32, tag="hT")
                            nc.vector.tensor_copy(hT[:], ps_t[:])
                            ps_xw = pp.tile([P, cout], F32, tag="psxw")
                            nc.tensor.matmul(ps_xw[:], lhsT=hT[:], rhs=wnext_s[:],
                                             start=True, stop=True)
                            y_and_xw(ps_xw, w, cout, dinv_s[:, w:w + 1],
                                     ys_next, xw_next, rows)
                        else:
                            gb = wp.tile([P, cfg.gslots], F32, tag="gb")
                            nc.vector.tensor_tensor(
                                out=gb[:],
                                in0=gslot_s[:, w:w + 1].broadcast_to([P, cfg.gslots]),
                                in1=iota_s[:, :cfg.gslots],
                                op=mybir.AluOpType.is_equal)
                            ps_p = pp1.tile([cfg.c3, cfg.gslots], F32, tag="psp")
                            nc.tensor.matmul(ps_p[:], lhsT=h[:], rhs=gb[:],
                                             start=True, stop=True)
                            nc.vector.tensor_tensor(
                                pool_acc[:], pool_acc[:], ps_p[:],
                                mybir.AluOpType.add)

            agg_layer(1, cfg.c1, cfg.c2, yfull64, xw_a, b1_s, w2_s, ys128, xw_b)
            nc.gpsimd.collective_compute(
                "AllGather", mybir.AluOpType.bypass, replica_groups=rg,
                ins=[ys128.opt()], outs=[yfull128.opt()])
            agg_layer(2, cfg.c2, cfg.c3, yfull128, xw_b, b2_s, w3_s, ys64, xw_a)
            nc.gpsimd.collective_compute(
                "AllGather", mybir.AluOpType.bypass, replica_groups=rg,
                ins=[ys64.opt()], outs=[yfull64.opt()])
            agg_layer(3, cfg.c3, None, yfull64, xw_a, b3_s, None, None, None)

            # ---- pooling finale ----
            nc.sync.dma_start(pool_sh[:], pool_acc[:])
            nc.gpsimd.collective_compute(
                "AllGather", mybir.AluOpType.bypass, replica_groups=rg,
                ins=[pool_sh.opt()], outs=[pool_ag.opt()])
            M = cp.tile([cfg.c3, cfg.n_graphs], F32)
            nc.vector.memset(M[:], 0.0)
            for c in range(n_cores):
                agc = wp.tile([cfg.c3, cfg.gslots], F32, tag="agc")
                nc.sync.dma_start(agc[:], pool_ag[c * cfg.c3:(c + 1) * cfg.c3, :])
                g0 = plan.g0[c]
                wdt = min(plan.gw[c], cfg.n_graphs - g0)
                nc.vector.tensor_tensor(M[:, g0:g0 + wdt], M[:, g0:g0 + wdt],
                                        agc[:, :wdt], mybir.AluOpType.add)
            M2 = cp.tile([cfg.c3, cfg.n_graphs], F32)
            nc.vector.tensor_tensor(M2[:], M[:], invcnt_s[:], mybir.AluOpType.mult)
            ps1 = pp1.tile([cfg.ch1, cfg.n_graphs], F32, tag="pst")
            nc.tensor.matmul(ps1[:], lhsT=wl1_s[:], rhs=M2[:], start=True, stop=True)
            g1 = cp.tile([cfg.ch1, cfg.n_graphs], F32)
            ts(g1[:], ps1[:], bl1_s[:, 0:1], mybir.AluOpType.add, 0.0,
               mybir.AluOpType.max)
            ps2 = pp1.tile([1, cfg.n_graphs], F32, tag="psp")
            nc.tensor.matmul(ps2[:], lhsT=wl2_s[:], rhs=g1[:], start=True, stop=True)
            osb = cp.tile([1, cfg.n_graphs], F32)
            ts(osb[:], ps2[:], bl2_s[:, 0:1], mybir.AluOpType.add)
            nc.sync.dma_start(out_d[:], osb[:])

    nc.compile()
    return nc


def make_in_maps(cfg, plan, shared, cores, x, W1, b1, W2, b2, W3, b3,
                 Wl1, bl1, Wl2, bl2):
    NS = cfg.ns
    x = np.asarray(x, dtype=np.float32)
    com = {
        "w1": np.asarray(W1, np.float32), "w2": np.asarray(W2, np.float32),
        "w3": np.asarray(W3, np.float32),
        "wl1": np.asarray(Wl1, np.float32), "wl2": np.asarray(Wl2, np.float32),
        "b1r": np.tile(np.asarray(b1, np.float32), (P, 1)),
        "b2r": np.tile(np.asarray(b2, np.float32), (P, 1)),
        "b3r": np.tile(np.asarray(b3, np.float32), (P, 1)),
        "bl1c": np.asarray(bl1, np.float32).reshape(-1, 1),
        "bl2c": np.asarray(bl2, np.float32).reshape(1, 1),
        "invcnt_r": shared["invcnt_r"], "iota_t": shared["iota_t"],
        "ident_t": np.eye(P, dtype=np.float32),
    }
    in_maps = []
    npad = cfg.nw * P
    for c in range(cfg.n_cores):
        m = dict(com)
        xs = np.zeros((cfg.in_ch, npad), dtype=np.float32)
        xs[:, :NS] = x[c * NS:(c + 1) * NS].T
        m["xT"] = xs
        m.update(cores[c])
        in_maps.append(m)
    return in_maps


_CACHE = {}


def _install_profile_hook():
    try:
        import antenv.axon_hooks  # noqa: F401
        return
    except ImportError:
        pass
    try:
        mod = types.ModuleType("antenv.axon_hooks")
        _h = [None]
        mod.set_axon_ntff_profile_hook = lambda h: _h.__setitem__(0, h)
        mod.get_axon_ntff_profile_hook = lambda: _h[0]
        sys.modules["antenv.axon_hooks"] = mod
        from trn_agent_boot.trn_boot import _ntff_profile_via_ctypes
        mod.set_axon_ntff_profile_hook(
            _ntff_profile_via_ctypes("/opt/axon/libaxon_pjrt.so"))
    except Exception:
        pass


def run(cfg, x, edge_index, batch, W1, b1, W2, b2, W3, b3, Wl1, bl1, Wl2, bl2,
        trace=False):
    plan, shared, cores = preprocess(cfg, edge_index, batch)
    key = ("prog", cfg.n_nodes, plan.nb_total, plan.idx_cols,
           tuple(plan.g0), tuple(plan.gw))
    if key not in _CACHE:
        _CACHE[key] = build_program(plan, cfg.n_cores)
    nc = _CACHE[key]
    in_maps = make_in_maps(cfg, plan, shared, cores, x, W1, b1, W2, b2,
                           W3, b3, Wl1, bl1, Wl2, bl2)
    if trace:
        _install_profile_hook()
    res = run_bass_kernel_spmd(nc, in_maps, list(range(cfg.n_cores)),
                               trace=trace)
    out = np.asarray(res.results[0]["out"]).reshape(-1)[:cfg.n_graphs]
    return out.astype(np.float32), res


def kernel(x, edge_index, batch, W1, b1, W2, b2, W3, b3, Wl1, bl1, Wl2, bl2):
    cfg = Cfg()
    out, _ = run(cfg, x, edge_index, batch, W1, b1, W2, b2, W3, b3,
                 Wl1, bl1, Wl2, bl2)
    return out

